# revision 39
# baseline (speedup 1.0000x reference)
"""Trainium2 Bass kernel for ViT-style multi-head attention with relative
position bias.

Problem (per full input):
  x        [8, 1024, 768] f32
  W_qkv    [768, 2304]    f32
  W_proj   [768, 768]     f32
  b_proj   [768]          f32
  bias_table [2047, 12]   f32
  rel_index  [1024, 1024] int32

Sharding: pure data parallel - one batch element per NeuronCore (B=8 over 8
cores), weights replicated. No collectives.

Per-core kernel (matmuls bf16, accum f32 in PSUM):
  - host pre-transposes x -> xT [C, N]; qT,kT computed in [d, n] layout,
    v in [n, d] layout - both directly from xT, no on-device transposes.
  - scores computed TRANSPOSED sT[j, i] = kT_chunk^T @ qT so the softmax'd
    matrix pT is already the PV matmul's moving operand. The two heads of a
    pair run concurrently at PE row groups 0/64 (K=64 row tiling).
  - rel-pos bias folded multiplicatively via a RESIDENT shifted table: the
    bias is Toeplitz (bias[j-i]), so E[p, i] for chunk jc equals
    s2[p, t, (896-128*jc)+i] where s2[p, t, z] = exp(table[1919-z+p, 2hp+t])
    is a [128, 2, 1920] bf16 SBUF tile per head-pair (host-precomputed, ~1MB
    DMA per pair, double-buffered). The bias multiply is then a plain SBUF
    tensor_tensor with an offset read - no per-tile DMA at all (the old
    version streamed a 25MB precomputed E table from HBM).
  - PV per (head, i-half): M=65 ones-row stationary [v | 1]; psum row 64 is
    the softmax denominator, captured by the bf16 eviction CAST. (Col-tiled
    or col-packed ACCUMULATION chains silently corrupt on this hardware -
    only independent col-tiled matmuls work - so PV keeps one chain per
    bank.) The reciprocal runs once per PAIR: the four bf16 den rows bounce
    through DRAM as one [128, 16] reshape, one reciprocal, and four
    broadcast reads with an f32->bf16 cast (SWDGE), so the normalize
    multiplies run at DVE 2x rate. The tail pair splits its dance into two
    it-halves and overlaps it with kc0..4 partial projections (borrowing
    idle score-psum banks).
  - proj: bias added during eviction as a DVE tensor_add against a
    partition-broadcast b_proj tile (the DVE is idle in the PE-bound tail,
    where K=1 bias matmuls would cost ~300ns of PE each). Split into a
    kc0..4 partial phase and a kc5+bias finish phase so partials fill the
    reciprocal-dance latency window.
  - HAM warmup: dummy matmuls at t=0 keep the PE clock at 2.4GHz through the
    input-DMA window (the old version spent 121us at half clock).

Emission is software-pipelined at head-pair granularity; all non-score
matmul groups are emitted at jc boundaries (never between the row-tiled
K=64 score matmuls of a pair - that corrupts in-flight PE state).
"""

import numpy as np
import ml_dtypes

B = 8
N = 1024
C = 768
H = 12
DH = 64
P = 128
KC = C // P          # 6 contraction chunks of 128 over C
NJ = N // P          # 8 chunks of 128 over the j (key) axis
NT = N // 512        # 2 tiles of 512 over the i (query) axis
HP = H // 2          # 6 head pairs
T5 = 512
SW = 1920            # shifted bias-table width
WARM = 8             # HAM warmup matmuls

_BUILT = {}


def _build_nc():
    from contextlib import ExitStack
    import concourse.bass as bass
    import concourse.mybir as mybir
    import concourse.tile as tile
    from concourse import bacc

    bf16 = mybir.dt.bfloat16
    f32 = mybir.dt.float32
    Exp = mybir.ActivationFunctionType.Exp
    Log = mybir.ActivationFunctionType.Ln

    nc = bacc.Bacc("TRN2", target_bir_lowering=False, debug=False)

    xT_d = nc.dram_tensor("xT", [P, KC, N], bf16, kind="ExternalInput")
    wqk_d = nc.dram_tensor("wqk", [P, HP, 2, KC, P], bf16, kind="ExternalInput")
    wv_d = nc.dram_tensor("wv", [P, KC, C], bf16, kind="ExternalInput")
    wp_d = nc.dram_tensor("wp", [P, KC, C], bf16, kind="ExternalInput")
    bp_d = nc.dram_tensor("bpb", [1, C], bf16, kind="ExternalInput")
    s2_d = nc.dram_tensor("s2", [HP, P, 2, SW], bf16, kind="ExternalInput")
    out_d = nc.dram_tensor("out", [N, C], f32, kind="ExternalOutput")

    with ExitStack() as ctx:
        tc = ctx.enter_context(tile.TileContext(nc))

        singles = ctx.enter_context(tc.tile_pool(name="singles", bufs=1))
        s2_pool = ctx.enter_context(tc.tile_pool(name="s2_pool", bufs=2))
        pt_pool = ctx.enter_context(tc.tile_pool(name="pt_pool", bufs=2))
        es_pool = ctx.enter_context(tc.tile_pool(name="es_pool", bufs=2))
        bc_pool = ctx.enter_context(tc.tile_pool(name="bc_pool", bufs=4))
        dc_pool = ctx.enter_context(tc.tile_pool(name="dc_pool", bufs=2))
        osb_pool = ctx.enter_context(tc.tile_pool(name="osb_pool", bufs=2))
        dram_pool = ctx.enter_context(
            tc.tile_pool(name="dram_pool", bufs=2, space="DRAM"))
        st_pool = ctx.enter_context(tc.tile_pool(name="st_pool", bufs=2))
        o_pool = ctx.enter_context(tc.tile_pool(name="o_pool", bufs=6))
        sc_ps = ctx.enter_context(tc.tile_pool(name="sc_ps", bufs=2, space="PSUM"))
        mm_ps = ctx.enter_context(tc.tile_pool(name="mm_ps", bufs=2, space="PSUM"))
        pv_ps = ctx.enter_context(tc.tile_pool(name="pv_ps", bufs=2, space="PSUM"))

        # ---- HAM warmup: dummy matmuls keep the PE clock hot during the
        # input-DMA window. They write a scratch psum tile that is never read.
        warm_sb = singles.tile([P, T5], bf16)
        # gpsimd memset: GpSimd's engine preamble is ~1.4us shorter than
        # DVE's, so the warmup matmuls (and the HAM un-throttle) start sooner
        nc.gpsimd.memset(warm_sb, 1.0)
        warm_ps = mm_ps.tile([P, T5], f32, tag="mm", name="warm_ps")
        for i in range(WARM):
            nc.tensor.matmul(warm_ps, warm_sb[:, 0:P], warm_sb,
                             start=True, stop=True)

        # ---- resident SBUF tensors; DMAs ordered compute-critical first ----
        # (all on the sync/SP HWDGE ring: the scalar/ACT ring stalls behind
        # the startup activation-table load, and gpsimd/SWDGE pays ~1us Q7
        # setup per transfer - both measured slower for these loads)
        wqk_sb = singles.tile([P, HP, 2, KC, P], bf16)
        nc.sync.dma_start(out=wqk_sb[:, 0], in_=wqk_d.ap()[:, 0])
        xT_sb = singles.tile([P, KC, N], bf16)
        for kc in range(KC):
            nc.sync.dma_start(out=xT_sb[:, kc], in_=xT_d.ap()[:, kc])
        s2_tiles = [None] * HP
        s2_tiles[0] = s2_pool.tile([P, 2, SW], bf16, tag="s2", name="s2_0")
        nc.sync.dma_start(out=s2_tiles[0], in_=s2_d.ap()[0])
        wv_sb = singles.tile([P, KC, C], bf16)
        nc.sync.dma_start(out=wv_sb, in_=wv_d.ap())
        for hp in range(1, HP):
            nc.sync.dma_start(out=wqk_sb[:, hp], in_=wqk_d.ap()[:, hp])
        wp_sb = singles.tile([P, KC, C], bf16)
        nc.sync.dma_start(out=wp_sb, in_=wp_d.ap())
        bpb_sb = singles.tile([P, C], bf16)
        bp_ap = bp_d.ap()
        nc.sync.dma_start(
            out=bpb_sb,
            in_=bass.AP(tensor=bp_ap.tensor, offset=bp_ap.offset,
                        ap=[[0, P], [1, C]]))
        ones1_sb = singles.tile([1, P], bf16)
        nc.vector.memset(ones1_sb, 1.0)
        ones64_sb = singles.tile([P, DH], bf16)   # ones row AT partition 64
        nc.vector.memset(ones64_sb[DH:DH + 1, :], 1.0)
        onesd_sb = singles.tile([P, 1], bf16)
        nc.vector.memset(onesd_sb, 1.0)

        qT_sb = singles.tile([P, HP, N], bf16)   # chunk hp = heads (2hp,2hp+1)
        kT_sb = singles.tile([P, HP, N], bf16)
        v_sb = singles.tile([P, NJ, H, DH + 1], bf16)  # col DH = ones
        nc.vector.memset(v_sb[:, :, :, DH:DH + 1], 1.0)
        outT_sb = singles.tile([P, HP, N], bf16)

        pts = [None] * HP
        dends = [None] * HP
        osbs = [[None] * 4 for _ in range(HP)]   # q = 0:(t0,i0) 1:(t1,i0) 2:(t0,i1) 3:(t1,i1)
        bcs = [[None] * 4 for _ in range(HP)]

        # ---- matmul group emitters (closures; emitted in pipelined order) --

        def v_group(nj, et):
            e0 = et * 384
            def emit():
                ps = mm_ps.tile([P, T5], f32, tag="mm", name=f"ps_v_{nj}_{et}")
                for kc in range(KC):
                    nc.tensor.matmul(
                        ps[:, :384],
                        xT_sb[:, kc, nj * P:(nj + 1) * P],
                        wv_sb[:, kc, e0:e0 + 384],
                        start=(kc == 0), stop=(kc == KC - 1),
                    )
                h0 = et * 6
                nc.vector.tensor_copy(
                    out=v_sb[:, nj, h0:h0 + 6, 0:DH],
                    in_=ps[:, :384].rearrange("p (h d) -> p h d", h=6),
                )
            return emit

        def qk_group(hp, which, it, use_sc=False):
            def emit():
                d = qT_sb if which == 0 else kT_sb
                if use_sc:
                    # prologue only: borrow the (still idle) score psum pool
                    # so the 4 qk(0) groups run back-to-back with no
                    # single-buffer eviction stalls
                    ps_t = sc_ps.tile([P, N], f32, tag="sc",
                                      name=f"psqk_{hp}_{which}_{it}")
                    ps = ps_t[:, 0:T5]
                else:
                    ps = mm_ps.tile([P, T5], f32, tag="mm",
                                    name=f"psqk_{hp}_{which}_{it}")
                for kc in range(KC):
                    nc.tensor.matmul(
                        ps,
                        wqk_sb[:, hp, which, kc, :],
                        xT_sb[:, kc, it * T5:(it + 1) * T5],
                        start=(kc == 0), stop=(kc == KC - 1),
                    )
                nc.vector.tensor_copy(
                    out=d[:, hp, it * T5:(it + 1) * T5], in_=ps)
            return emit

        # PV per (t, it): M=65 ones-row stationary [v | 1]; row 64 of the
        # psum tile is the softmax denominator. Single accumulation chain per
        # bank (col-tiled accumulation chains corrupt on this HW). The bf16
        # eviction CAST includes the denominator row, so the per-pair
        # reciprocal dance reads SBUF, not PSUM.
        def pv_group(hp, q):
            t, it = q % 2, q // 2
            def emit():
                pv = pv_ps.tile([DH + 1, T5], f32, tag="pv",
                                name=f"pv_{hp}_{q}")
                for jc in range(NJ):
                    nc.tensor.matmul(
                        pv,
                        v_sb[:, jc, 2 * hp + t, :],
                        pts[hp][:, jc, t, it * T5:(it + 1) * T5],
                        start=(jc == 0), stop=(jc == NJ - 1),
                    )
                osb = o_pool.tile([DH + 1, T5], bf16, tag="o",
                                  name=f"o_{hp}_{q}")
                osbs[hp][q] = osb
                if hp == HP - 1:
                    # tail pair: ScalarE is idle after the last exp, and the
                    # DVE is still draining bias multiplies - evicting via
                    # ScalarE starts the reciprocal-dance chain sooner
                    nc.scalar.copy(out=osb, in_=pv)
                else:
                    nc.vector.tensor_copy(out=osb, in_=pv)
                # stash this quarter's denominator row in the pair's DRAM
                # bounce buffer right away
                nc.sync.dma_start(
                    out=dends[hp][:, q * T5:(q + 1) * T5],
                    in_=osb[DH:DH + 1, :])
            return emit

        # reciprocal dance: reload stashed den rows as [128, f] (bf16),
        # reciprocal to f32, scatter back, broadcast-read [64, 512] bf16 bc
        # tiles. `halves` selects which it-halves to process (the tail does
        # them one at a time to overlap with PV/proj).
        def den_dance(hp, halves=(0, 1)):
            def emit():
                nq = 2 * len(halves)
                f = 4 * nq
                h0 = halves[0]
                dcol = dc_pool.tile([P, 16], bf16, tag="dcol",
                                    name=f"dcol_{hp}_{h0}")
                nc.sync.dma_start(
                    out=dcol[:, :f],
                    in_=dends[hp][:, 2 * h0 * T5:(2 * h0 + nq) * T5]
                    .rearrange("a (p f) -> (a p) f", p=P))
                rcol = dc_pool.tile([P, 16], f32, tag="rcol",
                                    name=f"rcol_{hp}_{h0}")
                nc.vector.reciprocal(out=rcol[:, :f], in_=dcol[:, :f])
                drb = dram_pool.tile([1, 4 * T5], f32, tag="drb",
                                     name=f"drb_{hp}_{h0}")
                nc.sync.dma_start(
                    out=drb[:, :nq * T5].rearrange("a (p f) -> (a p) f", p=P),
                    in_=rcol[:, :f])
                for qi in range(nq):
                    q = 2 * h0 + qi
                    bc = bc_pool.tile([DH, T5], bf16, tag="bc",
                                      name=f"bc_{hp}_{q}")
                    bcs[hp][q] = bc
                    src_ap = bass.AP(
                        tensor=drb.tensor,
                        offset=drb.offset + qi * T5,
                        ap=[[0, DH], [1, T5]])
                    nc.gpsimd.dma_start(out=bc, in_=src_ap)
            return emit

        # normalize + land in outT: t=0 heads write rows 0:64 directly;
        # t=1 heads go through a staging tile + partition-shift DMA.
        def ot_mul(hp, q):
            t, it = q % 2, q // 2
            def emit():
                if t == 0:
                    nc.vector.tensor_mul(
                        out=outT_sb[0:DH, hp, it * T5:(it + 1) * T5],
                        in0=osbs[hp][q][0:DH, :], in1=bcs[hp][q])
                else:
                    st = st_pool.tile([DH, T5], bf16, tag="st",
                                      name=f"st_{hp}_{q}")
                    nc.vector.tensor_mul(out=st, in0=osbs[hp][q][0:DH, :],
                                         in1=bcs[hp][q])
                    nc.gpsimd.dma_start(
                        out=outT_sb[DH:P, hp, it * T5:(it + 1) * T5],
                        in_=st)
            return emit

        proj_ps = {}

        def proj_partial(nj, borrow_sc=False, borrow_pv=False):
            # kc 0..4 accumulation - does not touch the last pair's outT rows,
            # so it can run while the tail's reciprocal dance is in flight.
            def emit():
                if borrow_sc:
                    sc_t = sc_ps.tile([P, N], f32, tag="sc",
                                      name=f"prb_{nj}")
                    pps = [sc_t[:, 0:384], sc_t[:, T5:T5 + 384]]
                elif borrow_pv:
                    pps = [pv_ps.tile([P, T5], f32, tag="pv",
                                      name=f"prv_{nj}_{et}")[:, :384]
                           for et in range(2)]
                else:
                    pps = [mm_ps.tile([P, T5], f32, tag="mm",
                                      name=f"pp_{nj}_{et}")[:, :384]
                           for et in range(2)]
                proj_ps[nj] = pps
                for et in range(2):
                    for kc in range(KC - 1):
                        nc.tensor.matmul(
                            pps[et],
                            outT_sb[:, kc, nj * P:(nj + 1) * P],
                            wp_sb[:, kc, et * 384:(et + 1) * 384],
                            start=(kc == 0), stop=False,
                        )
            return emit

        def proj_finish(nj):
            def emit():
                osb = osb_pool.tile([P, C], f32, tag="osb", name=f"osb_{nj}")
                kc = KC - 1
                for et in range(2):
                    pp = proj_ps[nj][et]
                    nc.tensor.matmul(
                        pp,
                        outT_sb[:, kc, nj * P:(nj + 1) * P],
                        wp_sb[:, kc, et * 384:(et + 1) * 384],
                        start=False, stop=True,
                    )
                    nc.vector.tensor_add(
                        out=osb[:, et * 384:(et + 1) * 384],
                        in0=pp,
                        in1=bpb_sb[:, et * 384:(et + 1) * 384])
                nc.sync.dma_start(
                    out=out_d.ap()[nj * P:(nj + 1) * P, :], in_=osb)
            return emit

        # tail-only reciprocal broadcast with zero DRAM round-trips: K=1
        # ones-matmul broadcasts the den row across 64 partitions into PSUM,
        # then a single fast-approx reciprocal gives 1/den in SBUF.
        def tail_norm(q):
            def emit():
                hp = HP - 1
                bps = pv_ps.tile([DH, T5], f32, tag="pv", name=f"bps_{q}")
                nc.tensor.matmul(bps, ones64_sb[DH:DH + 1, :],
                                 osbs[hp][q][DH:DH + 1, :],
                                 start=True, stop=True)
                rbc = dc_pool.tile([DH, T5], f32, tag="rbcf",
                                   name=f"rbc_{q}", bufs=4)
                nc.vector.reciprocal_approx_fast(out=rbc, in_=bps)
                bcs[hp][q] = rbc
            return emit

        def proj_group(nj, borrow_sc=False):
            def emit():
                proj_partial(nj, borrow_sc)()
                proj_finish(nj)()
            return emit

        # ---- scores phase: per jc, 4 row-paired K=64 matmuls, 2 exps, one
        # bias multiply reading the resident shifted table; slot groups are
        # emitted at jc boundaries only.
        def scores_phase(hp, slots):
            s2t = s2_tiles[hp]
            pt = pts[hp]
            for jc in range(NJ):
                sca = sc_ps.tile([P, N], f32, tag="sc", name=f"sc_{hp}_{jc}_0")
                scb = sc_ps.tile([P, N], f32, tag="sc", name=f"sc_{hp}_{jc}_1")
                for it in range(NT):
                    nc.tensor.matmul(
                        sca[:, it * T5:(it + 1) * T5],
                        kT_sb[0:64, hp, jc * P:(jc + 1) * P],
                        qT_sb[0:64, hp, it * T5:(it + 1) * T5],
                        start=True, stop=True,
                    )
                    nc.tensor.matmul(
                        scb[:, it * T5:(it + 1) * T5],
                        kT_sb[64:128, hp, jc * P:(jc + 1) * P],
                        qT_sb[64:128, hp, it * T5:(it + 1) * T5],
                        start=True, stop=True,
                    )
                es = es_pool.tile([P, 2, N], bf16, tag="es",
                                  name=f"es_{hp}_{jc}")
                off = 896 - P * jc
                nc.scalar.activation(out=es[:, 0], in_=sca, func=Exp)
                nc.vector.tensor_mul(
                    out=pt[:, jc, 0], in0=es[:, 0],
                    in1=s2t[:, 0, off:off + N])
                nc.scalar.activation(out=es[:, 1], in_=scb, func=Exp)
                nc.vector.tensor_mul(
                    out=pt[:, jc, 1], in0=es[:, 1],
                    in1=s2t[:, 1, off:off + N])
                for g in slots[jc]:
                    g()

        # ---- prologue: qk(0) via the idle score-psum pool (no evict
        # stalls), then the first 4 v groups.
        for it in range(NT):
            for which in range(2):
                qk_group(0, which, it, use_sc=True)()
        vg = [v_group(nj, et) for et in range(2) for nj in range(NJ)]
        for g in vg[:4]:
            g()
        vrest = vg[4:]
        # v-group filler schedule: et=0 groups all done during pair 0 (PV of
        # pair 0 runs at hp=1 jc5); et=1 done by pair 3 (PV of pair 3 at hp=4)
        v_sched = {0: (1, 3, 5, 7), 1: (1, 6, 7), 2: (1, 6), 3: (1, 6),
                   4: (1,)}

        # ---- main loop ----
        for hp in range(HP):
            slots = [[] for _ in range(NJ)]
            if hp + 1 < HP:
                def s2load(hpn=hp + 1):
                    s2_tiles[hpn] = s2_pool.tile([P, 2, SW], bf16, tag="s2",
                                                 name=f"s2_{hpn}")
                    nc.sync.dma_start(out=s2_tiles[hpn],
                                       in_=s2_d.ap()[hpn])
                slots[0].append(s2load)
                # next pair's q/k projections at jc 0,2,4,6
                qks = [qk_group(hp + 1, w, it)
                       for w in range(2) for it in range(NT)]
                for k, g in zip((0, 2, 4, 6), qks):
                    slots[k].append(g)
            # previous pair's four PV groups at jc 2..5, its dance right
            # after the 4th eviction, its first normalizes at jc 7
            dends[hp] = dram_pool.tile([1, 4 * T5], bf16, tag="dend",
                                       name=f"dend_{hp}")
            if hp > 0:
                for q in range(4):
                    slots[2 + q].append(pv_group(hp - 1, q))
                slots[5].append(den_dance(hp - 1))
                slots[7].append(ot_mul(hp - 1, 0))
                slots[7].append(ot_mul(hp - 1, 1))
            if hp > 1:
                slots[0].append(ot_mul(hp - 2, 2))
                slots[0].append(ot_mul(hp - 2, 3))
            # v-group fillers
            for k in v_sched.get(hp, ()):
                if vrest:
                    slots[k].append(vrest.pop(0))
            pts[hp] = pt_pool.tile([P, NJ, 2, N], bf16, tag="pt",
                                   name=f"pt_{hp}")
            scores_phase(hp, slots)

        # ---- tail ----
        pv_group(HP - 1, 0)()
        pv_group(HP - 1, 1)()
        den_dance(HP - 1, halves=(0,))()
        pv_group(HP - 1, 2)()
        pv_group(HP - 1, 3)()
        den_dance(HP - 1, halves=(1,))()
        ot_mul(HP - 2, 2)()
        ot_mul(HP - 2, 3)()
        # kc 0..4 partial projections keep the PE busy while the last pair's
        # reciprocals bounce through DRAM (nj 1,2 borrow idle score banks)
        proj_partial(0)()
        proj_partial(1, borrow_sc=True)()
        proj_partial(2, borrow_sc=True)()
        ot_mul(HP - 1, 0)()
        ot_mul(HP - 1, 1)()
        for nj in range(3):
            proj_finish(nj)()
        proj_group(3)()
        ot_mul(HP - 1, 2)()
        ot_mul(HP - 1, 3)()
        for nj in range(4, NJ):
            proj_group(nj)()

    nc.finalize()
    return nc


def _get_nc():
    if "nc" not in _BUILT:
        _BUILT["nc"] = _build_nc()
    return _BUILT["nc"]


def _prep_inputs(x, W_qkv, W_proj, b_proj, bias_table, rel_index):
    bf = ml_dtypes.bfloat16
    x = np.asarray(x, dtype=np.float32)
    W_qkv = np.asarray(W_qkv, dtype=np.float32)
    W_proj = np.asarray(W_proj, dtype=np.float32)
    b_proj = np.asarray(b_proj, dtype=np.float32)
    bias_table = np.asarray(bias_table, dtype=np.float32)
    rel_index = np.asarray(rel_index)

    xT = x.transpose(0, 2, 1).reshape(B, KC, P, N).transpose(0, 2, 1, 3)
    xT = np.ascontiguousarray(xT).astype(bf)                  # [B, P, KC, N]

    wq = W_qkv[:, :2 * C].copy()
    wq[:, :C] *= DH ** -0.5          # fold the attention scale into W_q
    # wqk[p, hp, w, kc, c] = wq[kc*128 + p, w*C + hp*128 + c]
    wqk = wq.reshape(KC, P, 2, HP, P).transpose(1, 3, 2, 0, 4)
    wqk = np.ascontiguousarray(wqk).astype(bf)
    # wv[p, kc, d] = W_qkv[kc*128 + p, 2C + d]
    wv = W_qkv[:, 2 * C:].reshape(KC, P, C).transpose(1, 0, 2)
    wv = np.ascontiguousarray(wv).astype(bf)
    wp = W_proj.reshape(KC, P, C).transpose(1, 0, 2)
    wp = np.ascontiguousarray(wp).astype(bf)
    bpb = np.ascontiguousarray(b_proj.reshape(1, C)).astype(bf)

    # shifted exp'd bias table: s2[hp, p, t, z] = exp(tbl[1919 - z + p, 2hp+t])
    # where tbl[m, h] = bias_table[m, h]; recovered through rel_index so the
    # kernel tracks the actual gather the reference performs.
    m = np.arange(2 * N - 1)
    i0 = np.maximum(0, (N - 1) - m)
    j0 = np.maximum(0, m - (N - 1))
    et = np.exp(bias_table[rel_index[i0, j0]])          # [2N-1, H]
    idx = (SW - 1) - np.arange(SW)[None, :] + np.arange(P)[:, None]
    s2 = et[idx]                                        # [P, SW, H]
    s2 = s2.transpose(2, 0, 1).reshape(HP, 2, P, SW).transpose(0, 2, 1, 3)
    s2 = np.ascontiguousarray(s2).astype(bf)

    shared = {"wqk": wqk, "wv": wv, "wp": wp, "bpb": bpb, "s2": s2}
    in_maps = []
    for b in range(B):
        mdict = dict(shared)
        mdict["xT"] = np.ascontiguousarray(xT[b])
        in_maps.append(mdict)
    return in_maps


def run(x, W_qkv, W_proj, b_proj, bias_table, rel_index, trace=False):
    """Returns (output [B, N, C] f32, exec_time_ns or None)."""
    from concourse.bass_utils import run_bass_kernel_spmd

    nc = _get_nc()
    in_maps = _prep_inputs(x, W_qkv, W_proj, b_proj, bias_table, rel_index)
    res = run_bass_kernel_spmd(nc, in_maps, core_ids=list(range(B)), trace=trace)
    out = np.stack([r["out"] for r in res.results]).astype(np.float32)
    return out, res.exec_time_ns


def kernel(x, W_qkv, W_proj, b_proj, bias_table, rel_index):
    out, _ = run(x, W_qkv, W_proj, b_proj, bias_table, rel_index, trace=False)
    return out
